# revision 38
# baseline (speedup 1.0000x reference)
"""Trainium2 Bass kernel for nn_Message_Passer (gnn_message_passing).

Reference computation:
    A = relu(edge_ij @ W + b)            # [B, E, 1024]
    messages = einsum("beij,bej->bei", A.reshape(B,E,32,32), node_j)

Strategy (8 NeuronCores, data-parallel over the flattened B*E edge dim),
v3: row-tiled matmul1. The PE reconfigures as two independent 64x128
tiles (contract dim 64 after dropping the bias ones-row; b is zeros in
this problem, and a bias-capable ACT-path variant exists as fallback).
Each core's 8192 edges split into stream A (first half) and stream B;
tile (0,0) computes A-stream A^T blocks while tile (64,0) concurrently
computes B-stream blocks from SBUF partitions 64-127 — halving matmul1
wall time on hardware (the serial cost model can't see this).

  - Host: edgeT2 [128, E/2] (rows 0-63 = stream-A features, 64-127 =
    stream-B), w2 [128, 1024] (W column-permuted, duplicated in both
    partition halves), ndP (node rows by pattern lo/hi, see below),
    sel3 [128, 128].
  - A-column map k = 128g + p -> (i, j) = (8*(g//2) + p//16,
    16*(g%2) + p%16): both banks of a pair share selector row-map
    m = 8q + p//16, so one sel lhsT serves both g's.
  - matmul1 per pair: 4 MMs of 256 cols (2 g x 2 streams); PSUM pair
    bank layout cols = [g0A g1A | g0B g1B] (tile (0,0) only touches
    bank alpha, (64,0) only bank beta -> no same-bank concurrency).
  - relu(+mult) per bank: in1 node factor n[16*(g%2) + p%16, e] comes
    from ndP patterns (lo: n[p%16], hi: n[16+p%16]); out pp reordered
    to per-g blocks [g0(A|B) | g1(A|B)] so each sel MM sees 512
    distinct edges.
  - j-reduction: 8 sel MMs/tile (one per g-block), shared lhsT per
    pair, accumulating into 4-tile-packed msg banks (strip rows
    32*(t%4) + i via tile_position col strips). Sel MMs for tile t
    issue as one batch during tile t+1 (2 PE tiling-mode switches per
    tile instead of 16).
  - Host extracts: strip row 32c+i, col e<256 -> A-edge 256*(4k+c)+e,
    col e>=256 -> B-edge E/2 + 256*(4k+c) + (e-256).
"""

import threading

import numpy as np
import ml_dtypes

import concourse.bass as bass
import concourse.mybir as mybir
import concourse.tile as tile
from concourse import bacc
from concourse.bass import ts, ds
from concourse.bass_utils import run_bass_kernel_spmd

N_CORES = 8
B, E_FULL, ND, ED = 16, 4096, 32, 64
EDGES = B * E_FULL            # 65536
E_CORE = EDGES // N_CORES     # 8192
E_HALF = E_CORE // 2          # 4096 (per stream)
ET = 512                      # edges per on-chip tile (256 A + 256 B)
HT = ET // 2                  # 256 edges per stream per tile
NT = E_CORE // ET             # 16 tiles
GT = 4                        # tiles per input-load group
GCOL = GT * HT                # 1024 edgeT2 columns per load group
NK = ND * ND                  # 1024 A-columns
F32 = mybir.dt.float32
F32R = mybir.dt.float32r
BF16 = mybir.dt.bfloat16

# Per-tile engine assignment for the relu(+mult) of the 4 PSUM bank-pairs:
# 'dve' = fused relu*mult STT on DVE (1x from PSUM); 'act' = relu on ACT
# (PSUM->SBUF bf16, GPSIMD cannot read PSUM) then 2x tensor_tensor mult on
# DVE; 'actp' = ACT relu then the mult on the otherwise-idle GPSIMD engine.
PAIR_MODES = [("dve", "act", "actp", "act"),
              ("dve", "act", "actp", "dve")]  # cycled by tile index


def _build_nc(repeat: int = 1, bias: bool = False):
    nc = bacc.Bacc("TRN2", target_bir_lowering=False, debug=False,
                   num_devices=N_CORES)
    edgeT_d = nc.dram_tensor("edgeT2", [128, E_HALF], F32R,
                             kind="ExternalInput")
    ndP_d = nc.dram_tensor("ndP", [128, 2 * E_CORE], BF16,
                           kind="ExternalInput")
    w_d = nc.dram_tensor("w2", [128, NK], F32R, kind="ExternalInput")
    sel_d = nc.dram_tensor("sel3", [128, 4 * ND], BF16, kind="ExternalInput")
    bias_d = nc.dram_tensor("bias2", [128, 8], F32, kind="ExternalInput")
    out_d = nc.dram_tensor("msg_raw", [128, E_CORE // 4], F32,
                           kind="ExternalOutput")

    with tile.TileContext(nc) as tc:
        with (
            tc.tile_pool(name="const", bufs=1) as constp,
            tc.tile_pool(name="edge", bufs=3) as edgep,
            tc.tile_pool(name="node", bufs=3) as nodep,
            tc.tile_pool(name="ar", bufs=10) as arp,
            tc.tile_pool(name="pp", bufs=10) as ppp,
            tc.tile_pool(name="mo", bufs=3) as mop,
            tc.tile_pool(name="apsum", bufs=3, space="PSUM") as apsum,
            tc.tile_pool(name="mpsum", bufs=2, space="PSUM") as mpsum,
        ):
            w_sb = constp.tile([128, NK], F32R, name="w_sb")
            sel_sb = constp.tile([128, 4 * ND], BF16, name="sel_sb")
            bias_sb = constp.tile([128, 8], F32, name="bias_sb")
            nc.sync.dma_start(out=bias_sb[:], in_=bias_d[:])
            w_chunks = [(0, 1), (1, 2), (2, 4), (4, 8)]
            sel_loaded = False

            # selector-matmul jobs lag ~1.5 tiles behind the mm1 stream and
            # are issued as one batch per tile, so the PE switches tiling
            # mode (64x128 mm1 <-> 128x32 sel strips) only twice per tile.
            sel_jobs = []

            def issue_sel(n):
                for _ in range(n):
                    if not sel_jobs:
                        return
                    mg_ap, blk, pp_, gl, st, sp = sel_jobs.pop(0)
                    mg_t, c4_, chunk, tail = mg_ap
                    nc.tensor.matmul(mg_t[32 * c4_:32 * (c4_ + 1), :],
                                     sel_sb[:, ts(blk, ND)],
                                     pp_[:, ts(gl, ET)],
                                     start=st, stop=sp,
                                     skip_group_check=True,
                                     tile_position=(0, 32 * c4_))
                    if sp and tail:
                        # final bank: evict + stream each strip as it lands
                        mo = mop.tile([32, ET], F32, name="mo_s")
                        nc.scalar.copy(mo[:], mg_t[32 * c4_:32 * (c4_ + 1), :])
                        nc.sync.dma_start(
                            out=out_d[32 * c4_:32 * (c4_ + 1), ts(chunk, ET)],
                            in_=mo[:])
                    elif sp and c4_ == 3:
                        # full 128-row bank: one evict + one DMA per 4 tiles
                        mo = mop.tile([128, ET], F32, name="mo")
                        nc.scalar.copy(mo[:], mg_t[:])
                        nc.gpsimd.dma_start(out=out_d[:, ts(chunk, ET)],
                                            in_=mo[:])

            def load_group(grp, first=False):
                # stream a 4-tile group of inputs; returns (ed_sb, nd_sb)
                ed_sb = edgep.tile([128, GCOL], F32R, name="ed_sb")
                nd_sb = nodep.tile([128, 4 * GCOL], BF16, name="nd_sb")
                ecols = ts(grp, GCOL)
                if first:
                    # startup: first edge chunk on SP; W chunks on the ACT
                    # queue so both HWDGE queues dispatch in parallel
                    nc.sync.dma_start(out=ed_sb[:, ts(0, HT)],
                                      in_=edgeT_d[:, ts(4 * grp, HT)])
                    for lo, hi in w_chunks:
                        nc.scalar.dma_start(out=w_sb[:, 128 * lo:128 * hi],
                                            in_=w_d[:, 128 * lo:128 * hi])
                    nc.sync.dma_start(out=ed_sb[:, HT:],
                                      in_=edgeT_d[:, 4 * grp * HT + HT:
                                                  (4 * grp + 4) * HT])
                else:
                    nc.sync.dma_start(out=ed_sb[:], in_=edgeT_d[:, ecols])
                # nd patterns: sb cols [pat*2048 + stream*1024 + e]
                for pat in range(2):
                    for s in range(2):
                        src0 = pat * E_CORE + s * E_HALF + grp * GCOL
                        dst0 = pat * 2 * GCOL + s * GCOL
                        nc.sync.dma_start(
                            out=nd_sb[:, dst0:dst0 + GCOL],
                            in_=ndP_d[:, src0:src0 + GCOL])
                return ed_sb, nd_sb

            mg = None
            ngrp = NT // GT
            pending_grp = None
            for tg in range(NT * repeat):
                t = tg % NT
                grp, loc = divmod(t, GT)
                if tg == 0:
                    ed_sb, nd_sb = load_group(0, first=True)
                elif loc == 0:
                    ed_sb, nd_sb = pending_grp
                if loc == 1 and tg - loc + GT < NT * repeat:
                    # prefetch the next group 3 tiles ahead of first use
                    pending_grp = load_group((grp + 1) % ngrp)
                tc0 = loc * HT          # tile-local edgeT2 column base
                if not sel_loaded:
                    nc.sync.dma_start(out=sel_sb[:], in_=sel_d[:])
                    sel_loaded = True

                # msg strip for this tile: rows 32c of the shared 4-tile bank
                c4 = t % 4
                if c4 == 0:
                    mg = mpsum.tile([128, ET], F32, name="mg")
                mg_ref = (mg, c4, t // 4, tg >= NT * repeat - 4)
                if bias:
                    modes = ("act", "act", "actp", "act")
                elif tg == 0:
                    # ACT is still loading its activation table
                    modes = ("dve", "act", "dve", "dve")
                else:
                    modes = PAIR_MODES[t % len(PAIR_MODES)]
                new_jobs = []
                pool_jobs = []
                ndv = nd_sb[:].rearrange("p (pat c) -> p pat c", pat=2)
                for q in range(4):
                    ap_t = apsum.tile([128, 2 * ET], F32, name="ap_t")
                    # 4 matmuls: 2 W-blocks x 2 streams; tile (0,0) covers
                    # stream A -> bank alpha only, (64,0) stream B -> bank
                    # beta only, so the concurrent tiles never share a bank
                    for gl in range(2):
                        g = 2 * q + gl
                        nc.tensor.matmul(ap_t[:, gl * HT:(gl + 1) * HT],
                                         w_sb[0:64, ts(g, 128)],
                                         ed_sb[0:64, tc0:tc0 + HT],
                                         start=True, stop=True,
                                         tile_position=(0, 0))
                        nc.tensor.matmul(ap_t[:, ET + gl * HT:
                                              ET + (gl + 1) * HT],
                                         w_sb[64:128, ts(g, 128)],
                                         ed_sb[64:128, tc0:tc0 + HT],
                                         start=True, stop=True,
                                         tile_position=(64, 0))
                    pp = ppp.tile([128, 2 * ET], BF16, name="pp")
                    for s in range(2):  # bank alpha (A) then beta (B)
                        in0 = ap_t[:, ts(s, ET)].rearrange(
                            "p (g e) -> p g e", g=2)
                        in1 = ndv[:, :, s * GCOL + tc0:
                                  s * GCOL + tc0 + HT]
                        # pp cols reordered to [512g + 256s + e] so each
                        # g-block holds 512 distinct edges for its sel MM
                        outv = pp[:].rearrange(
                            "p (g c) -> p g c", g=2)[:, :, s * HT:(s + 1) * HT]
                        if modes[q] == "dve":
                            nc.vector.scalar_tensor_tensor(
                                out=outv, in0=in0, scalar=0.0, in1=in1,
                                op0=mybir.AluOpType.max,
                                op1=mybir.AluOpType.mult)
                        else:
                            ar = arp.tile([128, 2, HT], BF16, name="ar")
                            if bias:
                                nc.scalar.activation(
                                    ar[:, 0, :], ap_t[:, s * ET:s * ET + HT],
                                    mybir.ActivationFunctionType.Relu,
                                    bias=bias_sb[:, 2 * q:2 * q + 1])
                                nc.scalar.activation(
                                    ar[:, 1, :],
                                    ap_t[:, s * ET + HT:(s + 1) * ET],
                                    mybir.ActivationFunctionType.Relu,
                                    bias=bias_sb[:, 2 * q + 1:2 * q + 2])
                            else:
                                nc.scalar.activation(
                                    ar[:], in0,
                                    mybir.ActivationFunctionType.Relu)
                            meng = (nc.gpsimd if modes[q] == "actp"
                                    else nc.vector)
                            meng.tensor_tensor(out=outv, in0=ar[:], in1=in1,
                                               op=mybir.AluOpType.mult)
                    dst = pool_jobs if modes[q] == "actp" else new_jobs
                    dst.append([mg_ref, q, pp, 0, False, False])
                    dst.append([mg_ref, q, pp, 1, False, False])
                # batch-issue earlier tiles' sel MMs after this tile's mm1s
                # (one tiling-mode switch each way per tile)
                issue_sel(4 if tg == 1 else 8)
                tile_jobs = new_jobs + pool_jobs
                tile_jobs[0][4] = True    # start accumulation on first issue
                tile_jobs[-1][5] = True   # stop on last issue
                sel_jobs.extend(tuple(j) for j in tile_jobs)

            # drain the remaining selector jobs
            issue_sel(len(sel_jobs))

    nc.compile()
    return nc


def _sel_matrix() -> np.ndarray:
    """sel3[p, 32q + m] = 1 iff m == 8q + p//16 (shared by both banks of
    pair q under the k-map i = 8*(g//2) + p//16, j = 16*(g%2) + p%16)."""
    sel = np.zeros((128, 4 * ND), dtype=np.float32)
    p = np.arange(128)
    for q in range(4):
        sel[p, 32 * q + 8 * q + p // 16] = 1.0
    return sel.astype(ml_dtypes.bfloat16)


def _kmap() -> np.ndarray:
    """perm[128g + p] = 32*i + j: original W column for packed column."""
    g = np.repeat(np.arange(8), 128)
    p = np.tile(np.arange(128), 8)
    i = 8 * (g // 2) + p // 16
    j = 16 * (g % 2) + p % 16
    return 32 * i + j


_LOCK = threading.Lock()
_NC = {}


def _get_nc(bias: bool = False):
    with _LOCK:
        if bias not in _NC:
            _NC[bias] = _build_nc(bias=bias)
    return _NC[bias]


def _prep_inputs(node_j, edge_ij, W, b):
    node_j = np.asarray(node_j, dtype=np.float32)
    edge_ij = np.asarray(edge_ij, dtype=np.float32)
    W = np.asarray(W, dtype=np.float32)
    b = np.asarray(b, dtype=np.float32)

    edge_flat = edge_ij.reshape(EDGES, ED)
    nodeT = node_j.reshape(EDGES, ND).T          # [32, EDGES]

    perm = _kmap()
    w2 = np.empty((128, NK), dtype=np.float32)
    w2[0:64] = W[:, perm]
    w2[64:128] = w2[0:64]

    # bias per packed column, one [128, 1] slice per bank g
    bias2 = np.ascontiguousarray(
        b[perm].reshape(8, 128).T).astype(np.float32)  # [128, 8]

    sel = _sel_matrix()
    p = np.arange(128)

    in_maps = []
    for c in range(N_CORES):
        lo = c * E_CORE
        ef = edge_flat[lo:lo + E_CORE]           # [8192, 64]
        edgeT2 = np.empty((128, E_HALF), dtype=np.float32)
        edgeT2[0:64] = ef[:E_HALF].T
        edgeT2[64:128] = ef[E_HALF:].T
        nt = nodeT[:, lo:lo + E_CORE]            # [32, 8192]
        ndP = np.empty((128, 2 * E_CORE), dtype=np.float32)
        ndP[:, :E_CORE] = nt[p % 16]             # pattern lo
        ndP[:, E_CORE:] = nt[16 + p % 16]        # pattern hi
        in_maps.append({
            "edgeT2": np.ascontiguousarray(edgeT2),
            "ndP": ndP.astype(ml_dtypes.bfloat16),
            "w2": w2,
            "sel3": sel,
            "bias2": bias2,
        })
    return in_maps


def _extract_msgT(msg_raw: np.ndarray) -> np.ndarray:
    """[128, E_core/4] packed bank image -> msgT [32, E_core].

    Chunk k strip c (rows 32c..32c+31) = tile 4k+c: cols 0-255 are
    A-edges 256*(4k+c)+e, cols 256-511 are B-edges E/2+256*(4k+c)+e."""
    nchunks = msg_raw.shape[1] // ET
    out = np.empty((ND, nchunks * 4 * ET), dtype=msg_raw.dtype)
    for k in range(nchunks):
        for c in range(4):
            tt = 4 * k + c
            blk = msg_raw[32 * c:32 * (c + 1), k * ET:(k + 1) * ET]
            out[:, HT * tt:HT * (tt + 1)] = blk[:, :HT]
            out[:, E_HALF + HT * tt:E_HALF + HT * (tt + 1)] = blk[:, HT:]
    return out


def kernel(node_j, edge_ij, W, b):
    use_bias = bool(np.any(np.asarray(b)))
    nc = _get_nc(bias=use_bias)
    in_maps = _prep_inputs(node_j, edge_ij, W, b)
    res = run_bass_kernel_spmd(nc, in_maps, core_ids=list(range(N_CORES)))
    msgT = np.concatenate(
        [_extract_msgT(res.results[c]["msg_raw"]) for c in range(N_CORES)],
        axis=1)  # [32, EDGES]
    return np.ascontiguousarray(msgT.T).reshape(B, E_FULL, ND)
